# revision 10
# baseline (speedup 1.0000x reference)
"""Trainium2 Bass kernel for BlockSelector (sparse-attention block-index masks).

Math (from the reference):
    i            = arange(S)
    cur_block    = i // block_size
    self_start   = cur_block * block_size             broadcast to [B,H,S]
    self_end     = min(i+1, (cur_block+1)*block_size) == i+1
    moba_valid   = topk < cur_block                   [B,H,S,K]
    moba_start   = where(valid, topk*block_size, 0)
    moba_end     = where(valid, (topk+1)*block_size, 0)

Only `topk_indices` is ever read; q/k/v are untouched.  self_start/self_end
are compile-time constants -> embedded in the NEFF and copied DRAM->DRAM.

Sharding: embarrassingly parallel over B*H = 32 (b,h) pairs -> 4 pairs per
core on 8 cores.  Layout: partition p = c*4 + j  (c = i//128 in [0,32),
j = pair in [0,4)), free dim = the 128*K contiguous elements of (i%128, k).
This gives 1536B-contiguous DMA descriptors per partition AND keeps
cur_block = c//4 = p//16 affine in p, so the validity mask is a single
scalar_tensor_tensor against an iota of (p - 15):
    valid  <=>  topk < p//16  <=>  16*topk < p - 15.

Raw bass (no TileContext): the whole kernel is a short static DAG; explicit
semaphores avoid Tile's kernel-tail drain (which exceeds the CoreV3
sync-wait slot limit here) and its multi-microsecond exit barrier.
"""

import numpy as np

import concourse.bass as bass
from concourse import mybir
from concourse.bass_utils import run_bass_kernel_spmd

B, H, S, K = 2, 16, 4096, 3
BLOCK = 512
NCORES = 8
PAIRS = B * H               # 32 (b,h) pairs
PPC = PAIRS // NCORES       # 4 pairs per core
P = 128                     # SBUF partitions
CH = 32                     # i-chunks of 128 per pair (S / 128)
M = S // CH                 # 128 consecutive seq positions per partition
F = M * K                   # 384 int32 elements per partition per pair

_cached = {}


def _self_consts():
    i = np.arange(S, dtype=np.int32)
    ss = np.broadcast_to((i // BLOCK) * BLOCK, (PPC, S)).copy()
    se = np.broadcast_to(i + 1, (PPC, S)).copy()
    return ss, se


def _build():
    op = mybir.AluOpType
    i32 = mybir.dt.int32
    u8 = mybir.dt.uint8

    nc = bass.Bass()
    tk_d = nc.dram_tensor("topk", [PPC, S, K], i32, kind="ExternalInput")
    ss_d = nc.dram_tensor("self_start", [PPC, S], i32, kind="ExternalOutput")
    se_d = nc.dram_tensor("self_end", [PPC, S], i32, kind="ExternalOutput")
    ms_d = nc.dram_tensor("moba_start", [PPC, S, K], i32, kind="ExternalOutput")
    me_d = nc.dram_tensor("moba_end", [PPC, S, K], i32, kind="ExternalOutput")
    mv_d = nc.dram_tensor("moba_valid", [PPC, S, K], u8, kind="ExternalOutput")

    ss_np, se_np = _self_consts()
    ss_c = nc.inline_tensor(ss_np, name="self_start_const")
    se_c = nc.inline_tensor(se_np, name="self_end_const")

    # DRAM view: [j, (c m), k] -> [c, j, (m k)]; pairs with an SBUF tile
    # whose partition dim is split (c j), i.e. partition p = c*4 + j.
    def part_view(dram):
        return dram[:].rearrange("j (c m) k -> c j (m k)", c=CH)


    with (
        nc.sbuf_tensor("tk_sb", [P, F], i32) as tk_sb,
        nc.sbuf_tensor("pb15", [P, F], i32) as pb15,
        nc.sbuf_tensor("valid", [P, F], i32) as valid,
        nc.sbuf_tensor("validu8", [P, F], u8) as validu8,
        nc.sbuf_tensor("mstart", [P, F], i32) as mstart,
        nc.sbuf_tensor("mend", [P, F], i32) as mend,
        nc.semaphore("s_in") as s_in,
        nc.semaphore("s_gp") as s_gp,
        nc.semaphore("s_v") as s_v,
        nc.semaphore("s_out") as s_out,
        nc.Block() as block,
    ):
        @block.gpsimd
        def _(gpsimd):
            # pb15[p, :] = p - 15
            gpsimd.iota(
                pb15[:], [[0, F]], base=-15, channel_multiplier=1
            ).then_inc(s_gp, 1)

        @block.vector
        def _(vector):
            vector.wait_ge(s_in, 16)
            vector.wait_ge(s_gp, 1)
            # valid = (tk*16) < (p-15)   <=>  tk < p//16
            vector.scalar_tensor_tensor(
                valid[:], tk_sb[:], 16, pb15[:], op0=op.mult, op1=op.is_lt
            ).then_inc(s_v, 1)
            vector.wait_ge(s_v, 1)
            # moba_start = (tk*512) * valid
            vector.scalar_tensor_tensor(
                mstart[:], tk_sb[:], BLOCK, valid[:], op0=op.mult, op1=op.mult
            ).then_inc(s_v, 1)
            vector.wait_ge(s_v, 2)
            # moba_end = (valid*512) + moba_start
            vector.scalar_tensor_tensor(
                mend[:], valid[:], BLOCK, mstart[:], op0=op.mult, op1=op.add
            ).then_inc(s_v, 1)
            vector.tensor_copy(validu8[:], valid[:]).then_inc(s_v, 1)

        @block.sync
        def _(sync):
            sync.dma_start(tk_sb[:], part_view(tk_d)).then_inc(s_in, 16)
            sync.wait_ge(s_v, 2)
            sync.dma_start(part_view(ms_d), mstart[:]).then_inc(s_out, 16)
            sync.wait_ge(s_v, 3)
            sync.dma_start(part_view(me_d), mend[:]).then_inc(s_out, 16)
            sync.wait_ge(s_out, 80)

        @block.scalar
        def _(scalar):
            scalar.dma_start(ss_d[:], ss_c[:]).then_inc(s_out, 16)
            scalar.dma_start(se_d[:], se_c[:]).then_inc(s_out, 16)
            scalar.wait_ge(s_v, 4)
            scalar.dma_start(part_view(mv_d), validu8[:]).then_inc(s_out, 16)
            scalar.wait_ge(s_out, 80)

    return nc


def _get_nc():
    if "nc" not in _cached:
        _cached["nc"] = _build()
    return _cached["nc"]


def kernel(q=None, k=None, v=None, topk_indices=None, query_block_indices=None,
           block_size=512, seq_len=4096, _run_kwargs=None, **_unused):
    tk = np.ascontiguousarray(np.asarray(topk_indices, dtype=np.int32))
    tk = tk.reshape(PAIRS, S, K)
    in_maps = [{"topk": tk[c * PPC:(c + 1) * PPC]} for c in range(NCORES)]

    nc = _get_nc()
    out = run_bass_kernel_spmd(nc, in_maps, list(range(NCORES)),
                               **(_run_kwargs or {}))
    res = out.results
    _cached["last_result"] = out

    def gather(name, shape, dtype):
        full = np.concatenate([np.asarray(res[c][name]) for c in range(NCORES)])
        return np.ascontiguousarray(full.reshape(shape).astype(dtype, copy=False))

    self_start = gather("self_start", (B, H, S), np.int32)
    self_end = gather("self_end", (B, H, S), np.int32)
    moba_start = gather("moba_start", (B, H, S, K), np.int32)
    moba_end = gather("moba_end", (B, H, S, K), np.int32)
    moba_valid = gather("moba_valid", (B, H, S, K), np.uint8).astype(bool)
    return (self_start, self_end, moba_start, moba_end, moba_valid)


# revision 11
# speedup vs baseline: 1.0088x; 1.0088x over previous
"""Trainium2 Bass kernel for BlockSelector (sparse-attention block-index masks).

Math (from the reference):
    i            = arange(S)
    cur_block    = i // block_size
    self_start   = cur_block * block_size             broadcast to [B,H,S]
    self_end     = min(i+1, (cur_block+1)*block_size) == i+1
    moba_valid   = topk < cur_block                   [B,H,S,K]
    moba_start   = where(valid, topk*block_size, 0)
    moba_end     = where(valid, (topk+1)*block_size, 0)

Only `topk_indices` is ever read; q/k/v are untouched.  self_start/self_end
are compile-time constants -> embedded in the NEFF and copied DRAM->DRAM.

Sharding: embarrassingly parallel over B*H = 32 (b,h) pairs -> 4 pairs per
core on 8 cores.  Layout: partition p = j*32 + c  (j = pair in [0,4),
c = i//128 in [0,32)), free dim = the 128*K contiguous elements of
(i%128, k).  That makes every big DMA a uniform 2-D access pattern
(partition stride = 1536B = the contiguous payload per partition), the
cheapest shape for HWDGE descriptor generation.  cur_block = (p%32)//4 is
not affine in p, so it ships as a tiny [128,1] float32 inline constant and
the validity mask is a single tensor_scalar per-partition compare
(the HW requires AP scalars to be f32; exact for these small ints).

Raw bass (no TileContext): the whole kernel is a short static DAG; explicit
semaphores avoid Tile's kernel-tail drain (which exceeds the CoreV3
sync-wait slot limit here) and its multi-microsecond exit barrier.
"""

import numpy as np

import concourse.bass as bass
from concourse import mybir
from concourse.bass_utils import run_bass_kernel_spmd

B, H, S, K = 2, 16, 4096, 3
BLOCK = 512
NCORES = 8
PAIRS = B * H               # 32 (b,h) pairs
PPC = PAIRS // NCORES       # 4 pairs per core
P = 128                     # SBUF partitions
CH = 32                     # i-chunks of 128 per pair (S / 128)
M = S // CH                 # 128 consecutive seq positions per partition
F = M * K                   # 384 int32 elements per partition per pair

_cached = {}


def _self_consts():
    i = np.arange(S, dtype=np.int32)
    ss = np.broadcast_to((i // BLOCK) * BLOCK, (PPC, S)).copy()
    se = np.broadcast_to(i + 1, (PPC, S)).copy()
    return ss, se


def _build():
    op = mybir.AluOpType
    i32 = mybir.dt.int32
    f32 = mybir.dt.float32
    u8 = mybir.dt.uint8

    nc = bass.Bass()
    tk_d = nc.dram_tensor("topk", [PPC, S, K], i32, kind="ExternalInput")
    ss_d = nc.dram_tensor("self_start", [PPC, S], i32, kind="ExternalOutput")
    se_d = nc.dram_tensor("self_end", [PPC, S], i32, kind="ExternalOutput")
    ms_d = nc.dram_tensor("moba_start", [PPC, S, K], i32, kind="ExternalOutput")
    me_d = nc.dram_tensor("moba_end", [PPC, S, K], i32, kind="ExternalOutput")
    mv_d = nc.dram_tensor("moba_valid", [PPC, S, K], u8, kind="ExternalOutput")

    ss_np, se_np = _self_consts()
    ss_c = nc.inline_tensor(ss_np, name="self_start_const")
    se_c = nc.inline_tensor(se_np, name="self_end_const")
    # cur_block per partition p = j*32 + c: (p % 32) // 4, as f32 (HW AP-scalar)
    cb_np = (((np.arange(P) % CH) * M) // BLOCK).astype(np.float32).reshape(P, 1)
    cb_c = nc.inline_tensor(cb_np, name="cur_block_const")

    # partition p = j*32 + c -> uniform 2D view [(j c), (m k)]
    def part_view(dram):
        return dram[:].rearrange("j (c m) k -> (j c) (m k)", c=CH)

    with (
        nc.sbuf_tensor("tk_sb", [P, F], i32) as tk_sb,
        nc.sbuf_tensor("cb_sb", [P, 1], f32) as cb_sb,
        nc.sbuf_tensor("valid", [P, F], i32) as valid,
        nc.sbuf_tensor("validu8", [P, F], u8) as validu8,
        nc.sbuf_tensor("mstart", [P, F], i32) as mstart,
        nc.sbuf_tensor("mend", [P, F], i32) as mend,
        nc.semaphore("s_in") as s_in,
        nc.semaphore("s_v") as s_v,
        nc.semaphore("s_out") as s_out,
        nc.Block(no_gpsimd_drain=True) as block,
    ):
        @block.vector
        def _(vector):
            vector.wait_ge(s_in, 32)
            # valid = tk < cur_block(p)
            vector.tensor_scalar(
                valid[:], tk_sb[:], cb_sb[:], None, op0=op.is_lt
            ).then_inc(s_v, 1)
            vector.wait_ge(s_v, 1)
            # moba_start = (tk*512) * valid
            vector.scalar_tensor_tensor(
                mstart[:], tk_sb[:], BLOCK, valid[:], op0=op.mult, op1=op.mult
            ).then_inc(s_v, 1)
            vector.wait_ge(s_v, 2)
            # moba_end = (valid*512) + moba_start
            vector.scalar_tensor_tensor(
                mend[:], valid[:], BLOCK, mstart[:], op0=op.mult, op1=op.add
            ).then_inc(s_v, 1)
            vector.tensor_copy(validu8[:], valid[:]).then_inc(s_v, 1)

        @block.sync
        def _(sync):
            sync.dma_start(tk_sb[:], part_view(tk_d)).then_inc(s_in, 16)
            sync.wait_ge(s_v, 2)
            sync.dma_start(part_view(ms_d), mstart[:]).then_inc(s_out, 16)
            sync.wait_ge(s_v, 3)
            sync.dma_start(part_view(me_d), mend[:]).then_inc(s_out, 16)
            sync.wait_ge(s_out, 80)

        @block.scalar
        def _(scalar):
            scalar.dma_start(cb_sb[:], cb_c[:]).then_inc(s_in, 16)
            scalar.dma_start(ss_d[:], ss_c[:]).then_inc(s_out, 16)
            scalar.dma_start(se_d[:], se_c[:]).then_inc(s_out, 16)
            scalar.wait_ge(s_v, 4)
            scalar.dma_start(part_view(mv_d), validu8[:]).then_inc(s_out, 16)
            scalar.wait_ge(s_out, 80)

    return nc


def _get_nc():
    if "nc" not in _cached:
        _cached["nc"] = _build()
    return _cached["nc"]


def kernel(q=None, k=None, v=None, topk_indices=None, query_block_indices=None,
           block_size=512, seq_len=4096, _run_kwargs=None, **_unused):
    tk = np.ascontiguousarray(np.asarray(topk_indices, dtype=np.int32))
    tk = tk.reshape(PAIRS, S, K)
    in_maps = [{"topk": tk[c * PPC:(c + 1) * PPC]} for c in range(NCORES)]

    nc = _get_nc()
    out = run_bass_kernel_spmd(nc, in_maps, list(range(NCORES)),
                               **(_run_kwargs or {}))
    res = out.results
    _cached["last_result"] = out

    def gather(name, shape, dtype):
        full = np.concatenate([np.asarray(res[c][name]) for c in range(NCORES)])
        return np.ascontiguousarray(full.reshape(shape).astype(dtype, copy=False))

    self_start = gather("self_start", (B, H, S), np.int32)
    self_end = gather("self_end", (B, H, S), np.int32)
    moba_start = gather("moba_start", (B, H, S, K), np.int32)
    moba_end = gather("moba_end", (B, H, S, K), np.int32)
    moba_valid = gather("moba_valid", (B, H, S, K), np.uint8).astype(bool)
    return (self_start, self_end, moba_start, moba_end, moba_valid)


# revision 14
# speedup vs baseline: 1.0230x; 1.0141x over previous
"""Trainium2 Bass kernel for BlockSelector (sparse-attention block-index masks).

Math (from the reference):
    i            = arange(S)
    cur_block    = i // block_size
    self_start   = cur_block * block_size             broadcast to [B,H,S]
    self_end     = min(i+1, (cur_block+1)*block_size) == i+1
    moba_valid   = topk < cur_block                   [B,H,S,K]
    moba_start   = where(valid, topk*block_size, 0)
    moba_end     = where(valid, (topk+1)*block_size, 0)

Only `topk_indices` is ever read; q/k/v are untouched.  self_start/self_end
are compile-time constants -> embedded in the NEFF and copied DRAM->DRAM.

Sharding: embarrassingly parallel over B*H = 32 (b,h) pairs -> 4 pairs per
core on 8 cores.  Layout: partition p = j*32 + c  (j = pair in [0,4),
c = i//128 in [0,32)), free dim = the 128*K contiguous elements of
(i%128, k).  That makes every big DMA a uniform 2-D access pattern
(partition stride = 1536B = the contiguous payload per partition), the
cheapest shape for HWDGE descriptor generation.  cur_block = (p%32)//4 is
not affine in p, so it ships as a tiny [128,1] float32 inline constant and
the validity mask is a single tensor_scalar per-partition compare
(the HW requires AP scalars to be f32; exact for these small ints).

Raw bass (no TileContext): the whole kernel is a short static DAG; explicit
semaphores avoid Tile's kernel-tail drain (which exceeds the CoreV3
sync-wait slot limit here) and its multi-microsecond exit barrier.
"""

import numpy as np

import concourse.bass as bass
from concourse import mybir
from concourse.bass_utils import run_bass_kernel_spmd

B, H, S, K = 2, 16, 4096, 3
BLOCK = 512
NCORES = 8
PAIRS = B * H               # 32 (b,h) pairs
PPC = PAIRS // NCORES       # 4 pairs per core
P = 128                     # SBUF partitions
CH = 32                     # i-chunks of 128 per pair (S / 128)
M = S // CH                 # 128 consecutive seq positions per partition
F = M * K                   # 384 int32 elements per partition per pair

_cached = {}


class _NoBarrierBlock(bass.BassBlock):
    """BassBlock without the exit-time all-engine barrier.

    The kernel is a one-shot static DAG: SP's final `wait_ge(s_out, 80)`
    already guarantees every output DMA has landed before SP retires, and
    no semaphore is ever reused, so the multi-microsecond end-of-block
    barrier butterfly is pure overhead.
    """

    def __exit__(self, exc_type, exc_val, exc_tb):
        if exc_type is not None:
            return
        for engine, last_body in self.last_body.items():
            with self.bass.body(
                last_body, parent=self.bass.cur_bb, allow_existing_parent=True
            ):
                engine.br(self.end_bb)
        self.bass.switch_bb(self.end_bb)


def _self_consts():
    i = np.arange(S, dtype=np.int32)
    ss = np.broadcast_to((i // BLOCK) * BLOCK, (PPC, S)).copy()
    se = np.broadcast_to(i + 1, (PPC, S)).copy()
    return ss, se


def _build():
    op = mybir.AluOpType
    i32 = mybir.dt.int32
    f32 = mybir.dt.float32
    u8 = mybir.dt.uint8

    nc = bass.Bass()
    tk_d = nc.dram_tensor("topk", [PPC, S, K], i32, kind="ExternalInput")
    ss_d = nc.dram_tensor("self_start", [PPC, S], i32, kind="ExternalOutput")
    se_d = nc.dram_tensor("self_end", [PPC, S], i32, kind="ExternalOutput")
    ms_d = nc.dram_tensor("moba_start", [PPC, S, K], i32, kind="ExternalOutput")
    me_d = nc.dram_tensor("moba_end", [PPC, S, K], i32, kind="ExternalOutput")
    mv_d = nc.dram_tensor("moba_valid", [PPC, S, K], u8, kind="ExternalOutput")

    ss_np, se_np = _self_consts()
    ss_c = nc.inline_tensor(ss_np, name="self_start_const")
    se_c = nc.inline_tensor(se_np, name="self_end_const")
    # cur_block per partition p = j*32 + c: (p % 32) // 4, as f32 (HW AP-scalar)
    cb_np = (((np.arange(P) % CH) * M) // BLOCK).astype(np.float32).reshape(P, 1)
    cb_c = nc.inline_tensor(cb_np, name="cur_block_const")

    # partition p = j*32 + c -> uniform 2D view [(j c), (m k)]
    def part_view(dram):
        return dram[:].rearrange("j (c m) k -> (j c) (m k)", c=CH)

    with (
        nc.sbuf_tensor("tk_sb", [P, F], i32) as tk_sb,
        nc.sbuf_tensor("cb_sb", [P, 1], f32) as cb_sb,
        nc.sbuf_tensor("valid", [P, F], i32) as valid,
        nc.sbuf_tensor("validu8", [P, F], u8) as validu8,
        nc.sbuf_tensor("mstart", [P, F], i32) as mstart,
        nc.sbuf_tensor("mend", [P, F], i32) as mend,
        nc.semaphore("s_in") as s_in,
        nc.semaphore("s_v") as s_v,
        nc.semaphore("s_out") as s_out,
        _NoBarrierBlock(nc, f"block_{nc.next_id()}") as block,
    ):
        nc.cur_block = block
        @block.vector
        def _(vector):
            vector.wait_ge(s_in, 32)
            # valid = tk < cur_block(p)
            vector.tensor_scalar(
                valid[:], tk_sb[:], cb_sb[:], None, op0=op.is_lt
            ).then_inc(s_v, 1)
            vector.wait_ge(s_v, 1)
            # moba_start = (tk*512) * valid
            vector.scalar_tensor_tensor(
                mstart[:], tk_sb[:], BLOCK, valid[:], op0=op.mult, op1=op.mult
            ).then_inc(s_v, 1)
            vector.wait_ge(s_v, 2)
            # moba_end = (valid*512) + moba_start
            vector.scalar_tensor_tensor(
                mend[:], valid[:], BLOCK, mstart[:], op0=op.mult, op1=op.add
            ).then_inc(s_v, 1)
            vector.tensor_copy(validu8[:], valid[:]).then_inc(s_v, 1)

        @block.sync
        def _(sync):
            sync.dma_start(tk_sb[:], part_view(tk_d)).then_inc(s_in, 16)
            sync.wait_ge(s_v, 2)
            sync.dma_start(part_view(ms_d), mstart[:]).then_inc(s_out, 16)
            sync.wait_ge(s_v, 3)
            sync.dma_start(part_view(me_d), mend[:]).then_inc(s_out, 16)
            sync.wait_ge(s_out, 80)

        @block.scalar
        def _(scalar):
            scalar.dma_start(cb_sb[:], cb_c[:]).then_inc(s_in, 16)
            scalar.dma_start(ss_d[:], ss_c[:]).then_inc(s_out, 16)
            scalar.dma_start(se_d[:], se_c[:]).then_inc(s_out, 16)
            scalar.wait_ge(s_v, 4)
            scalar.dma_start(part_view(mv_d), validu8[:]).then_inc(s_out, 16)
            scalar.wait_ge(s_out, 80)

    nc.cur_block = None
    return nc


def _get_nc():
    if "nc" not in _cached:
        _cached["nc"] = _build()
    return _cached["nc"]


def kernel(q=None, k=None, v=None, topk_indices=None, query_block_indices=None,
           block_size=512, seq_len=4096, _run_kwargs=None, **_unused):
    tk = np.ascontiguousarray(np.asarray(topk_indices, dtype=np.int32))
    tk = tk.reshape(PAIRS, S, K)
    in_maps = [{"topk": tk[c * PPC:(c + 1) * PPC]} for c in range(NCORES)]

    nc = _get_nc()
    out = run_bass_kernel_spmd(nc, in_maps, list(range(NCORES)),
                               **(_run_kwargs or {}))
    res = out.results
    _cached["last_result"] = out

    def gather(name, shape, dtype):
        full = np.concatenate([np.asarray(res[c][name]) for c in range(NCORES)])
        return np.ascontiguousarray(full.reshape(shape).astype(dtype, copy=False))

    self_start = gather("self_start", (B, H, S), np.int32)
    self_end = gather("self_end", (B, H, S), np.int32)
    moba_start = gather("moba_start", (B, H, S, K), np.int32)
    moba_end = gather("moba_end", (B, H, S, K), np.int32)
    moba_valid = gather("moba_valid", (B, H, S, K), np.uint8).astype(bool)
    return (self_start, self_end, moba_start, moba_end, moba_valid)
